# revision 16
# baseline (speedup 1.0000x reference)
"""Additive attention scorer: S[b,q,k] = sum_h wv[h] * tanh((qs@Wq)[b,q,h] + (ks@Wk)[b,k,h]).

Sharding: data-parallel over batch B=8 across the 8 NeuronCores (one batch
element per core). Per core:
  1. PE projects qT = Wq^T @ qs[b]^T and kT = Wk^T @ ks[b]^T  -> [H=128, 512]
     (H lands exactly on the 128 partitions). Inputs come in bf16 (the
     tanh-argument error budget tolerates it; halves DMA and keeps the
     matmuls single-pass -- fp32 matmuls decompose into 2 HW passes).
  2. For each query row lq: DVE builds X = kT + qT[:, lq] (per-partition
     scalar broadcast add) in bf16 (4x DVE mode), batched J rows per tile.
  3. ACT applies one big tanh over the batched tile (amortizes the ~224-cycle
     per-instruction overhead; ACT rate is dtype-independent so it is the
     hard bottleneck at ~1 elem/cycle/lane).
  4. PE reduces over H with a shifted-wv stationary trick: lhsT is a
     [128,128] window of a [128,255] bf16 tensor holding wv in column 127,
     so the single nonzero output row of each matmul lands on PSUM partition
     (lq mod 128); 128 matmuls accumulate one [128,512] fp32 output block.
  5. DVE copies PSUM->SBUF, DMA to DRAM.
"""

import numpy as np

B, LQ, LK, D, H = 8, 512, 512, 512, 128
P = 128           # SBUF partitions
ND = D // P       # contraction chunks for the projections
J = 16            # query rows batched per ACT instruction

_cache = {}


def _build():
    import concourse.bass as bass
    import concourse.tile as tile
    from concourse import bacc, mybir

    f32 = mybir.dt.float32
    bf16 = mybir.dt.bfloat16
    nc = bacc.Bacc("TRN2", target_bir_lowering=False, debug=False, num_devices=B)

    qsT = nc.dram_tensor("qsT", [D, LQ], bf16, kind="ExternalInput")
    ksT = nc.dram_tensor("ksT", [D, LK], bf16, kind="ExternalInput")
    Wq = nc.dram_tensor("Wq", [D, H], bf16, kind="ExternalInput")
    Wk = nc.dram_tensor("Wk", [D, H], bf16, kind="ExternalInput")
    wbig = nc.dram_tensor("wbig", [H, 2 * H - 1], bf16, kind="ExternalInput")
    out = nc.dram_tensor("out", [LQ, LK], bf16, kind="ExternalOutput")

    with tile.TileContext(nc) as tc:
        with (
            tc.tile_pool(name="const", bufs=1) as cpool,
            tc.tile_pool(name="load", bufs=ND) as lpool,
            tc.tile_pool(name="x", bufs=3) as xpool,
            tc.tile_pool(name="y", bufs=2) as ypool,
            tc.tile_pool(name="sout", bufs=2) as spool,
            tc.tile_pool(name="ppsum", bufs=2, space=bass.MemorySpace.PSUM) as ppool,
            tc.tile_pool(name="spsum", bufs=2, space=bass.MemorySpace.PSUM) as sppool,
        ):
            # Pre-warm the ACT tanh table set while the input DMAs are in
            # flight (the PSEUDO_LOAD_ACT_FUNC_SET costs ~2.7us once).
            warm = cpool.tile([P, 1], f32, tag="warm")
            nc.vector.memset(warm[:], 0.0)
            nc.scalar.activation(
                warm[:], warm[:], mybir.ActivationFunctionType.Tanh
            )

            wb = cpool.tile([P, 2 * H - 1], bf16, tag="wb")
            nc.gpsimd.dma_start(wb[:], wbig[:])

            # Projections: dst = W^T @ srcT accumulated over ND chunks of D.
            # One big DMA per input tensor -- a single InstDMACopy is split
            # across all 16 SDMA engines of its queue, so this parallelizes
            # better than per-chunk DMAs (which serialize on SWDGE issue).
            qT = cpool.tile([H, LQ], f32, tag="qT")
            kT = cpool.tile([H, LK], f32, tag="kT")
            srcs = {}
            # Small weight tensors on the gpsimd SWDGE ring (issued first so
            # descriptor gen overlaps the big HWDGE transfers).
            for name, dram in (("wk", Wk), ("wq", Wq)):
                t = lpool.tile([P, ND, H], bf16, tag=name)
                nc.gpsimd.dma_start(
                    t[:], dram[:, :].rearrange("(p c) k -> p c k", p=P)
                )
                srcs[name] = t
            # "(p c)" split: partition p holds DRAM rows 4p..4p+3, i.e. 4KB
            # contiguous per partition -> 4x bigger DMA descriptors. The
            # projection contracts over all of D either way, so the permuted
            # d-to-partition mapping is harmless (both operands use the same
            # mapping). Each tensor is split in half across the two HWDGE
            # queues (sync + scalar) for parallel streaming.
            for name, dram in (("ks", ksT), ("qs", qsT)):
                t = lpool.tile([P, ND, LK], bf16, tag=name)
                src_r = dram[:, :].rearrange("(p c) k -> p c k", p=P)
                nc.sync.dma_start(t[:, : ND // 2, :], src_r[:, : ND // 2, :])
                nc.scalar.dma_start(t[:, ND // 2 :, :], src_r[:, ND // 2 :, :])
                srcs[name] = t
            for sname, wname, dst in (("ks", "wk", kT), ("qs", "wq", qT)):
                ps = ppool.tile([H, LQ], f32, tag="proj")
                for c in range(ND):
                    nc.tensor.matmul(
                        ps[:],
                        srcs[wname][:, c, :],
                        srcs[sname][:, c, :],
                        start=(c == 0),
                        stop=(c == ND - 1),
                    )
                nc.vector.tensor_copy(dst[:], ps[:])

            tanh = mybir.ActivationFunctionType.Tanh
            NBLK = LQ // P  # output row-blocks
            GPB = P // J    # groups per block
            for blk in range(NBLK):
                sp = sppool.tile([P, LK], f32, tag="spsum")
                for g in range(GPB):
                    gi = blk * GPB + g
                    y = ypool.tile([P, J * LK], bf16, tag="y")
                    x = xpool.tile([P, J * LK], f32, tag="x")
                    for j in range(J):
                        lq = blk * P + g * J + j
                        nc.vector.tensor_scalar_add(
                            x[:, j * LK : (j + 1) * LK], kT[:], qT[:, lq : lq + 1]
                        )
                    if gi == 0:
                        # Finer ACT granularity for the very first group so
                        # the scalar engine starts ~4us earlier.
                        for s in range(4):
                            sl = slice(s * 4 * LK, (s + 1) * 4 * LK)
                            nc.scalar.activation(y[:, sl], x[:, sl], tanh)
                    elif gi == NBLK * GPB - 1:
                        # Split the final group so its reduce-matmuls overlap
                        # the tail of the tanh work.
                        h = J // 2 * LK
                        nc.scalar.activation(y[:, :h], x[:, :h], tanh)
                        nc.scalar.activation(y[:, h:], x[:, h:], tanh)
                    else:
                        nc.scalar.activation(y[:], x[:], tanh)
                    for j in range(J):
                        r = g * J + j
                        nc.tensor.matmul(
                            sp[:],
                            wb[:, H - 1 - r : 2 * H - 1 - r],
                            y[:, j * LK : (j + 1) * LK],
                            start=(r == 0),
                            stop=(r == P - 1),
                        )
                s_sb = spool.tile([P, LK], bf16, tag="sout")
                nc.vector.tensor_copy(s_sb[:], sp[:])
                nc.sync.dma_start(out[blk * P : (blk + 1) * P, :], s_sb[:])

    nc.compile()
    return nc


def _in_maps(qs, ks, Wq, Wk, wv):
    import ml_dtypes

    bf = ml_dtypes.bfloat16
    wbig = np.zeros((H, 2 * H - 1), np.float32)
    wbig[:, H - 1] = wv
    wbig = wbig.astype(bf)
    Wq_b = np.ascontiguousarray(Wq, dtype=np.float32).astype(bf)
    Wk_b = np.ascontiguousarray(Wk, dtype=np.float32).astype(bf)
    qs = np.asarray(qs)
    ks = np.asarray(ks)
    maps = []
    for b in range(B):
        maps.append(
            {
                "qsT": np.ascontiguousarray(qs[b].T).astype(bf),
                "ksT": np.ascontiguousarray(ks[b].T).astype(bf),
                "Wq": Wq_b,
                "Wk": Wk_b,
                "wbig": wbig,
            }
        )
    return maps


def run(qs, ks, Wq, Wk, wv, trace=False):
    from concourse.bass_utils import run_bass_kernel_spmd

    if "nc" not in _cache:
        _cache["nc"] = _build()
    res = run_bass_kernel_spmd(
        _cache["nc"],
        _in_maps(qs, ks, Wq, Wk, wv),
        core_ids=list(range(B)),
        trace=trace,
    )
    outs = np.stack([np.asarray(res.results[i]["out"]) for i in range(B)], axis=0)
    return outs.astype(np.float32), res


def kernel(qs, ks, Wq, Wk, wv):
    out, _ = run(qs, ks, Wq, Wk, wv, trace=False)
    return out


# revision 17
# speedup vs baseline: 1.1903x; 1.1903x over previous
"""Additive attention scorer: S[b,q,k] = sum_h wv[h] * tanh((qs@Wq)[b,q,h] + (ks@Wk)[b,k,h]).

Sharding: data-parallel over batch B=8 across the 8 NeuronCores (one batch
element per core). Per core:
  1. PE projects qT = Wq^T @ qs[b]^T and kT = Wk^T @ ks[b]^T  -> [H=128, 512]
     (H lands exactly on the 128 partitions). Inputs come in bf16 (the
     tanh-argument error budget tolerates it; halves DMA and keeps the
     matmuls single-pass -- fp32 matmuls decompose into 2 HW passes).
  2. For each query row lq: DVE builds X = kT + qT[:, lq] (per-partition
     scalar broadcast add) in bf16 (4x DVE mode), batched J rows per tile.
  3. ACT applies one big tanh over the batched tile (amortizes the ~224-cycle
     per-instruction overhead; ACT rate is dtype-independent so it is the
     hard bottleneck at ~1 elem/cycle/lane).
  4. PE reduces over H with a shifted-wv stationary trick: lhsT is a
     [128,128] window of a [128,255] bf16 tensor holding wv in column 127,
     so the single nonzero output row of each matmul lands on PSUM partition
     (lq mod 128); 128 matmuls accumulate one [128,512] fp32 output block.
  5. DVE copies PSUM->SBUF, DMA to DRAM.
"""

import numpy as np

B, LQ, LK, D, H = 8, 512, 512, 512, 128
P = 128           # SBUF partitions
ND = D // P       # contraction chunks for the projections
J = 16            # query rows batched per ACT instruction

_cache = {}


def _build():
    import concourse.bass as bass
    import concourse.tile as tile
    from concourse import bacc, mybir

    f32 = mybir.dt.float32
    bf16 = mybir.dt.bfloat16
    nc = bacc.Bacc("TRN2", target_bir_lowering=False, debug=False, num_devices=B)

    qsT = nc.dram_tensor("qsT", [D, LQ], bf16, kind="ExternalInput")
    ksT = nc.dram_tensor("ksT", [D, LK], bf16, kind="ExternalInput")
    Wq = nc.dram_tensor("Wq", [D, H], bf16, kind="ExternalInput")
    Wk = nc.dram_tensor("Wk", [D, H], bf16, kind="ExternalInput")
    wbig = nc.dram_tensor("wbig", [H, 2 * H - 1], bf16, kind="ExternalInput")
    out = nc.dram_tensor("out", [LQ, LK], bf16, kind="ExternalOutput")

    with tile.TileContext(nc) as tc:
        with (
            tc.tile_pool(name="const", bufs=1) as cpool,
            tc.tile_pool(name="load", bufs=ND) as lpool,
            tc.tile_pool(name="x", bufs=3) as xpool,
            tc.tile_pool(name="y", bufs=2) as ypool,
            tc.tile_pool(name="sout", bufs=2) as spool,
            tc.tile_pool(name="ppsum", bufs=2, space=bass.MemorySpace.PSUM) as ppool,
            tc.tile_pool(name="spsum", bufs=2, space=bass.MemorySpace.PSUM) as sppool,
        ):
            # Pre-warm the ACT tanh table set while the input DMAs are in
            # flight (the PSEUDO_LOAD_ACT_FUNC_SET costs ~2.7us once).
            warm = cpool.tile([P, 1], f32, tag="warm")
            nc.vector.memset(warm[:], 0.0)
            nc.scalar.activation(
                warm[:], warm[:], mybir.ActivationFunctionType.Tanh
            )

            wb = cpool.tile([P, 2 * H - 1], bf16, tag="wb")
            nc.gpsimd.dma_start(wb[:], wbig[:])

            # Projections: dst = W^T @ srcT accumulated over ND chunks of D.
            # One big DMA per input tensor -- a single InstDMACopy is split
            # across all 16 SDMA engines of its queue, so this parallelizes
            # better than per-chunk DMAs (which serialize on SWDGE issue).
            qT = cpool.tile([H, LQ], f32, tag="qT")
            kT = cpool.tile([H, LK], f32, tag="kT")
            srcs = {}
            # Small weight tensors on the gpsimd SWDGE ring (issued first so
            # descriptor gen overlaps the big HWDGE transfers).
            for name, dram in (("wk", Wk), ("wq", Wq)):
                t = lpool.tile([P, ND, H], bf16, tag=name)
                nc.gpsimd.dma_start(
                    t[:], dram[:, :].rearrange("(p c) k -> p c k", p=P)
                )
                srcs[name] = t
            # "(p c)" split: partition p holds DRAM rows 4p..4p+3, i.e. 4KB
            # contiguous per partition -> 4x bigger DMA descriptors. The
            # projection contracts over all of D either way, so the permuted
            # d-to-partition mapping is harmless (both operands use the same
            # mapping). Both ride the sync HWDGE queue: each InstDMACopy is
            # split across all 16 SDMA engines, and the scalar queue must be
            # left alone (it starves once ACT saturates).
            for name, dram in (("ks", ksT), ("qs", qsT)):
                t = lpool.tile([P, ND, LK], bf16, tag=name)
                src_r = dram[:, :].rearrange("(p c) k -> p c k", p=P)
                nc.sync.dma_start(t[:], src_r[:])
                srcs[name] = t
            for sname, wname, dst in (("ks", "wk", kT), ("qs", "wq", qT)):
                ps = ppool.tile([H, LQ], f32, tag="proj")
                for c in range(ND):
                    nc.tensor.matmul(
                        ps[:],
                        srcs[wname][:, c, :],
                        srcs[sname][:, c, :],
                        start=(c == 0),
                        stop=(c == ND - 1),
                    )
                nc.vector.tensor_copy(dst[:], ps[:])

            tanh = mybir.ActivationFunctionType.Tanh
            NBLK = LQ // P  # output row-blocks
            GPB = P // J    # groups per block
            for blk in range(NBLK):
                sp = sppool.tile([P, LK], f32, tag="spsum")
                for g in range(GPB):
                    gi = blk * GPB + g
                    y = ypool.tile([P, J * LK], bf16, tag="y")
                    x = xpool.tile([P, J * LK], f32, tag="x")
                    for j in range(J):
                        lq = blk * P + g * J + j
                        nc.vector.tensor_scalar_add(
                            x[:, j * LK : (j + 1) * LK], kT[:], qT[:, lq : lq + 1]
                        )
                    if gi == 0:
                        # Finer ACT granularity for the very first group so
                        # the scalar engine starts ~4us earlier.
                        for s in range(4):
                            sl = slice(s * 4 * LK, (s + 1) * 4 * LK)
                            nc.scalar.activation(y[:, sl], x[:, sl], tanh)
                    elif gi == NBLK * GPB - 1:
                        # Split the final group so its reduce-matmuls overlap
                        # the tail of the tanh work.
                        h = J // 2 * LK
                        nc.scalar.activation(y[:, :h], x[:, :h], tanh)
                        nc.scalar.activation(y[:, h:], x[:, h:], tanh)
                    else:
                        nc.scalar.activation(y[:], x[:], tanh)
                    for j in range(J):
                        r = g * J + j
                        nc.tensor.matmul(
                            sp[:],
                            wb[:, H - 1 - r : 2 * H - 1 - r],
                            y[:, j * LK : (j + 1) * LK],
                            start=(r == 0),
                            stop=(r == P - 1),
                        )
                s_sb = spool.tile([P, LK], bf16, tag="sout")
                nc.vector.tensor_copy(s_sb[:], sp[:])
                nc.sync.dma_start(out[blk * P : (blk + 1) * P, :], s_sb[:])

    nc.compile()
    return nc


def _in_maps(qs, ks, Wq, Wk, wv):
    import ml_dtypes

    bf = ml_dtypes.bfloat16
    wbig = np.zeros((H, 2 * H - 1), np.float32)
    wbig[:, H - 1] = wv
    wbig = wbig.astype(bf)
    Wq_b = np.ascontiguousarray(Wq, dtype=np.float32).astype(bf)
    Wk_b = np.ascontiguousarray(Wk, dtype=np.float32).astype(bf)
    qs = np.asarray(qs)
    ks = np.asarray(ks)
    maps = []
    for b in range(B):
        maps.append(
            {
                "qsT": np.ascontiguousarray(qs[b].T).astype(bf),
                "ksT": np.ascontiguousarray(ks[b].T).astype(bf),
                "Wq": Wq_b,
                "Wk": Wk_b,
                "wbig": wbig,
            }
        )
    return maps


def run(qs, ks, Wq, Wk, wv, trace=False):
    from concourse.bass_utils import run_bass_kernel_spmd

    if "nc" not in _cache:
        _cache["nc"] = _build()
    res = run_bass_kernel_spmd(
        _cache["nc"],
        _in_maps(qs, ks, Wq, Wk, wv),
        core_ids=list(range(B)),
        trace=trace,
    )
    outs = np.stack([np.asarray(res.results[i]["out"]) for i in range(B)], axis=0)
    return outs.astype(np.float32), res


def kernel(qs, ks, Wq, Wk, wv):
    out, _ = run(qs, ks, Wq, Wk, wv, trace=False)
    return out
